# revision 2
# baseline (speedup 1.0000x reference)
"""DecoderLSTM Trainium2 kernel.

Reference computation (see problem):
    embedded = emb_table[sequence]                       [B,S,E]
    const = enc_h[0] @ W_hh.T + b_hh + b_ih              [B,4H]
    emb_gates = einsum('bse,ge->sbg', embedded, W_ih[:, :E])
    scan over s: gates = emb_gates[s] + prev_h @ W_h.T + const
                 i,f,g,o = split(gates); c = sig(f)*c0 + sig(i)*tanh(g)
                 h = sig(o)*tanh(c)
    out[b, s*H:(s+1)*H] = h_s[b]

Sharding: data-parallel over batch (4 rows per core, 8 cores); the time
scan stays local per core. Device layout keeps the 4H gate dim on SBUF
partitions (16 chunks of 128) and the 4 local batch lanes in the free
dim, so the recurrent matmul is 64 stationary-weight [128,128]@[128,4]
fp16 matmuls per step and all pointwise work runs on [128,16] tiles.
"""

import sys

sys.path.insert(0, "/opt/trn_rl_repo")

import numpy as np

import concourse.bass as bass
import concourse.tile as tile
from concourse import bacc, mybir
from concourse.masks import make_identity

VOCAB, E, H = 50257, 512, 512
B, S_FULL = 32, 512
NCORES = 8
BL = B // NCORES          # batch rows per core
G4 = 4 * H                # 2048 gate dim
MCH = G4 // 128           # 16 gate chunks
KCH = H // 128            # 4 contraction chunks
F32 = mybir.dt.float32
F16 = mybir.dt.float16
I32 = mybir.dt.int32

# gate reorder: [i, f, o, g] so sigmoid covers one contiguous block
PERM = np.concatenate([np.arange(0, 1024), np.arange(1536, 2048), np.arange(1024, 1536)])


def _lstm_kernel(tc, aps, n_steps):
    nc = tc.nc
    emb_tab = aps["emb_tab"]
    w_eT = aps["w_eT"]
    w_hT = aps["w_hT"]
    w_hhT = aps["w_hhT"]
    bias_l = aps["bias_l"]
    c0_l = aps["c0_l"]
    h0_l = aps["h0_l"]
    idx_l = aps["idx_l"]
    hist_d = aps["hist"]

    n_tok = n_steps * BL
    tok_chunks = (n_tok + 511) // 512       # 512-token GEMM chunks
    hist_chunks = n_steps // 16

    sig = mybir.ActivationFunctionType.Sigmoid
    tanh = mybir.ActivationFunctionType.Tanh

    with tc.tile_pool(name="wts", bufs=1) as wts:
        w_e_sb = wts.tile([128, KCH, G4], F32, tag="w_e")
        w_h_sb = wts.tile([128, KCH, G4], F16, tag="w_h")
        w_hh_sb = wts.tile([128, KCH, G4], F32, tag="w_hh")
        for k in range(KCH):
            nc.sync.dma_start(w_e_sb[:, k, :], w_eT[128 * k:128 * (k + 1), :])
            nc.sync.dma_start(w_h_sb[:, k, :], w_hT[128 * k:128 * (k + 1), :])
            nc.sync.dma_start(w_hh_sb[:, k, :], w_hhT[128 * k:128 * (k + 1), :])
        bias_sb = wts.tile([128, MCH], F32, tag="bias")
        nc.sync.dma_start(bias_sb[:], bias_l[:])
        c0_sb = wts.tile([128, MCH], F32, tag="c0")
        nc.sync.dma_start(c0_sb[:], c0_l[:])
        h0_sb = wts.tile([128, KCH, BL], F32, tag="h0")
        nc.sync.dma_start(h0_sb[:], h0_l[:])
        idx_sb = wts.tile([128, n_tok // 128], I32, tag="idx")
        nc.sync.dma_start(idx_sb[:], idx_l[:, : n_tok // 128])
        ident = wts.tile([128, 128], F32, tag="ident")
        make_identity(nc, ident[:])
        # emb_const: (emb_gates + const) transposed, fp16: [g_p, m, s, b]
        emb_sb = wts.tile([128, MCH, n_steps, BL], F16, tag="emb")
        const_sb = wts.tile([128, MCH, BL], F32, tag="const")

        # ---- phase 1: const = h0 @ W_hh.T + bias -------------------------
        with tc.tile_pool(name="cps", bufs=2, space="PSUM") as cps:
            for m in range(MCH):
                pc = cps.tile([128, BL], F32, tag="pc")
                for k in range(KCH):
                    nc.tensor.matmul(
                        pc[:],
                        lhsT=w_hh_sb[:, k, 128 * m:128 * (m + 1)],
                        rhs=h0_sb[:, k, :],
                        start=(k == 0),
                        stop=(k == KCH - 1),
                    )
                nc.vector.tensor_scalar_add(const_sb[:, m, :], pc[:], bias_sb[:, m:m + 1])

        # ---- phase 2: gather + transpose + GEMM --------------------------
        n_sc = n_tok // 128                    # 128-token sub-chunks
        with (
            tc.tile_pool(name="gath", bufs=2) as gath,
            tc.tile_pool(name="gps", bufs=2, space="PSUM") as gps,
        ):
            for j in range(tok_chunks):
                scs = list(range(4 * j, min(4 * (j + 1), n_sc)))
                w = 128 * len(scs)
                embedded = gath.tile([128, 4, E], F32, tag="embedded")
                for i, sc in enumerate(scs):
                    nc.gpsimd.indirect_dma_start(
                        out=embedded[:, i, :],
                        out_offset=None,
                        in_=emb_tab[:, :],
                        in_offset=bass.IndirectOffsetOnAxis(
                            ap=idx_sb[:, sc:sc + 1], axis=0
                        ),
                    )
                embT = gath.tile([128, KCH, 512], F32, tag="embT")
                for i in range(len(scs)):
                    for k in range(KCH):
                        pt = gps.tile([128, 128], F32, tag="pt")
                        nc.tensor.transpose(
                            pt[:], embedded[:, i, 128 * k:128 * (k + 1)], ident[:]
                        )
                        nc.scalar.copy(embT[:, k, 128 * i:128 * (i + 1)], pt[:])
                for m in range(MCH):
                    pg = gps.tile([128, 512], F32, tag="pg")
                    for k in range(KCH):
                        nc.tensor.matmul(
                            pg[:, :w],
                            lhsT=w_e_sb[:, k, 128 * m:128 * (m + 1)],
                            rhs=embT[:, k, :w],
                            start=(k == 0),
                            stop=(k == KCH - 1),
                        )
                    cb = const_sb[:, m, :]
                    const_bcast = bass.AP(
                        tensor=cb.tensor,
                        offset=cb.offset,
                        ap=[cb.ap[0], [0, w // BL], cb.ap[1]],
                    )
                    nc.vector.scalar_tensor_tensor(
                        out=emb_sb[:, m, 128 * j:128 * j + w // BL, :],
                        in0=pg[:, :w].rearrange("p (s b) -> p s b", b=BL),
                        scalar=1.0,
                        in1=const_bcast,
                        op0=mybir.AluOpType.mult,
                        op1=mybir.AluOpType.add,
                    )

        # ---- phase 3: the scan -------------------------------------------
        with (
            tc.tile_pool(name="zq", bufs=2, space="PSUM") as zq,
            tc.tile_pool(name="hq", bufs=3) as hq,
            tc.tile_pool(name="sp", bufs=3) as sp,
            tc.tile_pool(name="hp", bufs=2) as hp,
        ):
            h_prev = hq.tile([128, KCH, BL], F16, tag="h")
            nc.vector.memset(h_prev[:], 0.0)
            hist_t = None
            for t in range(n_steps):
                z = zq.tile([128, MCH * BL], F32, tag="z")
                for m in range(MCH):
                    for k in range(KCH):
                        nc.tensor.matmul(
                            z[:, BL * m:BL * (m + 1)],
                            lhsT=w_h_sb[:, k, 128 * m:128 * (m + 1)],
                            rhs=h_prev[:, k, :],
                            start=(k == 0),
                            stop=(k == KCH - 1),
                        )
                zv = z[:].rearrange("p (m b) -> p m b", b=BL)
                nc.vector.tensor_add(zv[:, 0:12, :], zv[:, 0:12, :], emb_sb[:, 0:12, t, :])
                nc.vector.tensor_add(zv[:, 12:16, :], zv[:, 12:16, :], emb_sb[:, 12:16, t, :])
                sfo = sp.tile([128, 48], F32, tag="sfo")
                nc.scalar.activation(sfo[:], z[:, 0:48], sig)
                gg = sp.tile([128, 16], F32, tag="gg")
                nc.scalar.activation(gg[:], z[:, 48:64], tanh)
                t1 = sp.tile([128, 16], F32, tag="t1")
                nc.vector.tensor_mul(t1[:], sfo[:, 0:16], gg[:])
                t2 = sp.tile([128, 16], F32, tag="t2")
                nc.vector.tensor_mul(t2[:], sfo[:, 16:32], c0_sb[:])
                cc = sp.tile([128, 16], F32, tag="cc")
                nc.vector.tensor_add(cc[:], t1[:], t2[:])
                tc_ = sp.tile([128, 16], F32, tag="tc")
                nc.scalar.activation(tc_[:], cc[:], tanh)
                h_new = hq.tile([128, KCH, BL], F16, tag="h")
                nc.vector.tensor_mul(
                    h_new[:].rearrange("p k b -> p (k b)"), sfo[:, 32:48], tc_[:]
                )
                if t % 16 == 0:
                    hist_t = hp.tile([128, 16, 16], F32, tag="hist")
                nc.vector.tensor_mul(hist_t[:, t % 16, :], sfo[:, 32:48], tc_[:])
                if t % 16 == 15:
                    nc.sync.dma_start(hist_d[t // 16], hist_t[:])
                h_prev = h_new


def _build(n_steps):
    nc = bacc.Bacc(
        "TRN2",
        target_bir_lowering=False,
        debug=False,
        enable_asserts=True,
        num_devices=NCORES,
    )
    n_tok = n_steps * BL
    aps = {
        "emb_tab": nc.dram_tensor("emb_tab", [VOCAB, E], F32, kind="ExternalInput").ap(),
        "w_eT": nc.dram_tensor("w_eT", [E, G4], F32, kind="ExternalInput").ap(),
        "w_hT": nc.dram_tensor("w_hT", [H, G4], F16, kind="ExternalInput").ap(),
        "w_hhT": nc.dram_tensor("w_hhT", [H, G4], F32, kind="ExternalInput").ap(),
        "bias_l": nc.dram_tensor("bias_l", [128, MCH], F32, kind="ExternalInput").ap(),
        "c0_l": nc.dram_tensor("c0_l", [128, MCH], F32, kind="ExternalInput").ap(),
        "h0_l": nc.dram_tensor("h0_l", [128, KCH, BL], F32, kind="ExternalInput").ap(),
        "idx_l": nc.dram_tensor("idx_l", [128, n_tok // 128], I32, kind="ExternalInput").ap(),
        "hist": nc.dram_tensor(
            "hist", [n_steps // 16, 128, 16, 16], F32, kind="ExternalOutput"
        ).ap(),
    }
    with tile.TileContext(nc) as tc:
        _lstm_kernel(tc, aps, n_steps)
    nc.compile()
    return nc


_CACHE = {}


def _get_nc(n_steps):
    if n_steps not in _CACHE:
        _CACHE[n_steps] = _build(n_steps)
    return _CACHE[n_steps]


def make_in_maps(sequence, enc_h, enc_c, emb_table, W_ih, W_hh, b_ih, b_hh, n_steps):
    """Host-side sharding + weight relayout. Returns list of 8 per-core input maps."""
    sequence = np.asarray(sequence)
    enc_h = np.asarray(enc_h, dtype=np.float32)
    enc_c = np.asarray(enc_c, dtype=np.float32)
    emb_table = np.ascontiguousarray(np.asarray(emb_table, dtype=np.float32))
    W_ih = np.asarray(W_ih, dtype=np.float32)
    W_hh = np.asarray(W_hh, dtype=np.float32)
    bias = (np.asarray(b_ih, dtype=np.float32) + np.asarray(b_hh, dtype=np.float32))

    W_ihP = W_ih[PERM]
    W_hhP = W_hh[PERM]
    biasP = bias[PERM]
    w_eT = np.ascontiguousarray(W_ihP[:, :E].T)                      # [512, 2048] f32
    w_hT = np.ascontiguousarray(W_ihP[:, E:].T).astype(np.float16)   # [512, 2048] f16
    w_hhT = np.ascontiguousarray(W_hhP.T)                            # [512, 2048] f32
    bias_l = np.ascontiguousarray(biasP.reshape(MCH, 128).T)         # [128, 16]

    in_maps = []
    for c in range(NCORES):
        bsl = slice(BL * c, BL * (c + 1))
        seq = sequence[bsl, :n_steps]                     # [4, n_steps]
        ids = np.ascontiguousarray(seq.T).reshape(-1)     # tok = s*BL + b
        idx_l = np.ascontiguousarray(
            ids.reshape(-1, 128).T
        ).astype(np.int32)                                # [128, n_tok/128]
        h0 = enc_h[0, bsl]                                # [4, 512]
        h0_l = np.ascontiguousarray(h0.T.reshape(KCH, 128, BL).transpose(1, 0, 2))
        c0 = enc_c[0, bsl]
        c0_l = np.ascontiguousarray(
            c0.T.reshape(KCH, 128, BL).transpose(1, 0, 2).reshape(128, MCH)
        )
        in_maps.append(
            {
                "emb_tab": emb_table,
                "w_eT": w_eT,
                "w_hT": w_hT,
                "w_hhT": w_hhT,
                "bias_l": bias_l,
                "c0_l": c0_l,
                "h0_l": h0_l,
                "idx_l": idx_l,
            }
        )
    return in_maps


def assemble_output(hists, n_steps):
    """hists: list of 8 per-core [n_steps/16, 128, 16, 16] f32 arrays."""
    out = np.empty((B, n_steps * H), dtype=np.float32)
    for c in range(NCORES):
        arr = hists[c].reshape(n_steps // 16, 128, 16, 4, BL)
        # [chunk, p, t, m, b] -> [b, chunk, t, m, p] -> [BL, n_steps*H]
        out[BL * c:BL * (c + 1)] = np.ascontiguousarray(
            arr.transpose(4, 0, 2, 3, 1)
        ).reshape(BL, n_steps * H)
    return out


def kernel(sequence, enc_out, enc_h, enc_c, emb_table, W_ih, W_hh, b_ih, b_hh):
    from concourse.bass_utils import run_bass_kernel_spmd

    n_steps = S_FULL
    nc = _get_nc(n_steps)
    in_maps = make_in_maps(
        sequence, enc_h, enc_c, emb_table, W_ih, W_hh, b_ih, b_hh, n_steps
    )
    res = run_bass_kernel_spmd(nc, in_maps, core_ids=list(range(NCORES)))
    return assemble_output([r["hist"] for r in res.results], n_steps)
